# revision 12
# baseline (speedup 1.0000x reference)
"""GAT (2-layer, 8-head) + graph pooling + MLP on 8 TRN2 NeuronCores.

Single merged SPMD program (one dispatch per call) with on-device collectives:
  - shard destination nodes (and incident edges) across the 8 cores
  - each core computes the [h | alpha_src] gather-table rows for ITS node
    shard only; an AllGather assembles the full 50176-row table in DRAM
    (replaces the host-mediated h1 exchange of the 3-program version)
  - per layer: edge pass with dma_gather by src (640-wide bf16 rows),
    attention ex = exp(leakyrelu(as+ad)), one-hot S matmuls for
    segment-softmax sum/den per 128-dst block
  - layer-2 epilogue pools nodes into 3 local 128-graph windows (one-hot
    matmuls vs graph-local ids), windows land in a [2048, 512] global graph
    grid via dma_gather (handles the per-core graph offset g0), then a
    ReduceScatter hands core c the pooled rows [256c, 256c+256)
  - each core runs the small MLP on its 256 graphs -> out [1, 256]

Host <-> device traffic is minimized further by caching device-resident
inputs across calls (validated with full np.array_equal before reuse).
"""
import os
import time
import numpy as np
import ml_dtypes
from contextlib import ExitStack

os.environ.setdefault("JAX_PLATFORMS", "")  # allow axon platform auto-detect

LAST_TIMES = {}

import concourse.bass as bass
import concourse.bacc as bacc
import concourse.mybir as mybir
import concourse.tile as tile
from concourse.masks import make_identity

BF = ml_dtypes.bfloat16

N = 50000
E = 800000
NF = 32
H = 8
F = 64
HF = 512
G = 2000
NEG = 0.2
NCORES = 8
NSH = N // NCORES            # 6250 dst nodes per core
NBLK = (NSH + 127) // 128    # 49 blocks
NSHP = NBLK * 128            # 6272
NPADN = NCORES * NSHP        # 50176 padded-global rows (core-major)
HALF = 4 * NSHP              # 25088 = cores 0-3 -> table A (int16 idx range)
NWIN = 3                     # 128-graph windows per core (local range < 384)
WROWS = 512                  # winbuf rows: 384 window rows + zero pad
ZROW = 400                   # a guaranteed-zero winbuf row for out-of-range
GRID = 2048                  # global graph grid rows (>= G, 16 tiles of 128)
GSH = GRID // NCORES         # 256 graphs per core after ReduceScatter

F32 = mybir.dt.float32
BF16 = mybir.dt.bfloat16
I16 = mybir.dt.int16

_cache = {}


# ---------------------------------------------------------------- host prep
def _preprocess(edge_index, batch):
    src = np.concatenate([edge_index[0], np.arange(N, dtype=np.int64)]).astype(np.int64)
    dst = np.concatenate([edge_index[1], np.arange(N, dtype=np.int64)]).astype(np.int64)
    core = dst // NSH
    dloc = (dst - core * NSH).astype(np.int64)
    blk = dloc // 128
    # padded-global table row of the source node (core-major, stride NSHP)
    spad = (src // NSH) * NSHP + (src % NSH)
    tab = (spad >= HALF).astype(np.int64)

    # group key per edge: (core, blk, tab)
    key = (core * NBLK + blk) * 2 + tab
    order = np.argsort(key, kind="stable")
    src_s, dst_s, key_s = spad[order], dst[order], key[order]
    counts = np.bincount(key_s, minlength=NCORES * NBLK * 2).reshape(NCORES, NBLK * 2)

    # uniform chunk counts across cores
    K = np.ceil(counts.max(axis=0) / 128.0).astype(np.int64)  # [NBLK*2]
    TOTCH = int(K.sum())
    choff = np.concatenate([[0], np.cumsum(K)])  # chunk offset per group

    # per-core flat edge slot arrays [TOTCH*128]
    srci = np.zeros((NCORES, TOTCH * 128), np.int16)
    dsti = np.zeros((NCORES, TOTCH * 128), np.int16)
    dstl = np.full((NCORES, TOTCH * 128), -1.0, np.float32)

    gstart = np.concatenate([[0], np.cumsum(counts.reshape(-1))[:-1]])
    gs = gstart.reshape(NCORES, NBLK * 2)
    for c in range(NCORES):
        for g in range(NBLK * 2):
            n = counts[c, g]
            if n == 0:
                continue
            s0 = gs[c, g]
            es, ed = src_s[s0:s0 + n], dst_s[s0:s0 + n]
            o0 = choff[g] * 128
            t = g & 1
            srci[c, o0:o0 + n] = (es - t * HALF).astype(np.int16)
            dl = (ed - c * NSH).astype(np.int64)
            dsti[c, o0:o0 + n] = dl.astype(np.int16)
            dstl[c, o0:o0 + n] = (dl - (g // 2) * 128).astype(np.float32)

    # gather runs: per group, runs of <=8 chunks
    gathers = []  # (tab, chunk0, nch)
    for g in range(NBLK * 2):
        k = int(K[g])
        c0 = int(choff[g])
        while k > 0:
            nch = min(k, 8)
            gathers.append((g & 1, c0, nch))
            c0 += nch
            k -= nch

    def wrap16(v):  # [n] -> [128, n//16] column-major wrap, replicated
        n = v.shape[0]
        return np.tile(v.reshape(n // 16, 16).T, (8, 1)).astype(np.int16)

    idx_src = [np.concatenate(
        [wrap16(srci[c, c0 * 128:(c0 + nch) * 128]) for (_, c0, nch) in gathers], axis=1)
        for c in range(NCORES)]
    idx_dst = [np.concatenate(
        [wrap16(dsti[c, c0 * 128:(c0 + nch) * 128]) for (_, c0, nch) in gathers], axis=1)
        for c in range(NCORES)]
    dstl_t = [dstl[c].reshape(TOTCH, 128).T.copy() for c in range(NCORES)]

    # block boundaries in chunk space: block b covers chunks [choff[2b], choff[2b+2])
    blk_first = [int(choff[2 * b]) for b in range(NBLK)]
    blk_last = [int(choff[2 * b + 2]) - 1 for b in range(NBLK)]

    # graph-local window values per (lane, block, window)
    g0 = [int(batch[c * NSH]) for c in range(NCORES)]
    glw = []
    for c in range(NCORES):
        gmax = int(batch[(c + 1) * NSH - 1]) if c < NCORES - 1 else int(batch[N - 1])
        assert gmax - g0[c] < NWIN * 128, "graph-local id range exceeds windows"
        a = np.full((128, NBLK * NWIN), -1.0, np.float32)
        for b in range(NBLK):
            nn = min(128, NSH - b * 128)
            nodes = c * NSH + b * 128 + np.arange(nn)
            gl = batch[nodes] - g0[c]
            for w in range(NWIN):
                a[:nn, b * NWIN + w] = gl - 128 * w
        glw.append(a)

    # grid gather indices: grid row r <- winbuf row (r - g0) if in window
    # range else the zeroed row ZROW
    gridx = []
    for c in range(NCORES):
        r = np.arange(GRID, dtype=np.int64) - g0[c]
        idx = np.where((r >= 0) & (r < NWIN * 128), r, ZROW).astype(np.int16)
        gridx.append(np.concatenate(
            [np.tile(idx[t * 128:(t + 1) * 128].reshape(8, 16).T, (8, 1))
             for t in range(GRID // 128)], axis=1))

    return dict(TOTCH=TOTCH, gathers=gathers, blk_first=blk_first, blk_last=blk_last,
                idx_src=idx_src, idx_dst=idx_dst, dstl=dstl_t, glw=glw, g0=g0,
                gridx=gridx)


def _wcat(Wmat, a_vec):
    # [fin, H*F] weight + per-head attention vec -> [fin, H] alpha weight
    fin = Wmat.shape[0]
    Wr = Wmat.reshape(fin, H, F)
    return np.einsum("fhk,hk->fh", Wr, a_vec)


# ------------------------------------------------------------- device build
def _edge_pass(nc, tc, ctx, meta, tabA, tabB, adtab, layer, consts, epil):
    """Shared edge-processing pass. epil(b, num_ps, den_ps) emits the block
    epilogue after the block's last chunk."""
    IC_off = 0
    sb = ctx.enter_context(tc.tile_pool(name=f"eg{layer}", bufs=3))
    sbm = ctx.enter_context(tc.tile_pool(name=f"em{layer}", bufs=6))
    psN = ctx.enter_context(tc.tile_pool(name=f"pn{layer}", bufs=2, space="PSUM"))
    psD = ctx.enter_context(tc.tile_pool(name=f"pd{layer}", bufs=2, space="PSUM"))

    iota_bf = consts["iota_bf"]
    dstl_sb = consts["dstl_sb"]
    isrc_sb = consts["isrc_sb"]
    idst_sb = consts["idst_sb"]

    num_ps = den_ps = None
    cur_blk = -1
    for (t, c0, nch) in meta["gathers"]:
        n = nch * 128
        cols = nch * 8
        gt = sb.tile([128, nch, 640], BF16, tag="maing")
        nc.gpsimd.dma_gather(
            out_ap=gt[:], in_ap=(tabA if t == 0 else tabB),
            idxs_ap=isrc_sb[:, IC_off:IC_off + cols],
            num_idxs=n, num_idxs_reg=n, elem_size=640)
        adt = sb.tile([128, nch, 128], BF16, tag="adg")
        nc.gpsimd.dma_gather(
            out_ap=adt[:], in_ap=adtab[:],
            idxs_ap=idst_sb[:, IC_off:IC_off + cols],
            num_idxs=n, num_idxs_reg=n, elem_size=128)
        IC_off += cols

        e_st = sb.tile([128, nch * 8], F32, tag="est")
        for j in range(nch):
            nc.vector.tensor_tensor(
                out=e_st[:, 8 * j:8 * j + 8], in0=gt[:, j, 512:520],
                in1=adt[:, j, 0:8], op=mybir.AluOpType.add)
        t_sc = sb.tile([128, nch * 8], F32, tag="esc")
        nc.vector.tensor_scalar(out=t_sc[:], in0=e_st[:], scalar1=NEG, scalar2=None,
                                op0=mybir.AluOpType.mult)
        nc.vector.tensor_tensor(out=e_st[:], in0=e_st[:], in1=t_sc[:],
                                op=mybir.AluOpType.max)
        ex_st = sb.tile([128, nch * 8], BF16, tag="exs")
        nc.scalar.activation(ex_st[:], e_st[:], mybir.ActivationFunctionType.Exp)

        for j in range(nch):
            ch = c0 + j
            if num_ps is None or ch > meta["blk_last"][cur_blk]:
                cur_blk += 1
                num_ps = psN.tile([128, 512], F32, tag="nps")
                den_ps = psD.tile([128, 8], F32, tag="dps")
            S = sbm.tile([128, 128], BF16, tag="S")
            nc.vector.tensor_scalar(
                out=S[:], in0=iota_bf[:], scalar1=dstl_sb[:, ch:ch + 1],
                scalar2=None, op0=mybir.AluOpType.is_equal)
            msg = sbm.tile([128, 512], BF16, tag="msg")
            nc.vector.tensor_tensor(
                out=msg[:].rearrange("p (h f) -> p h f", h=H),
                in0=gt[:, j, 0:512].rearrange("p (h f) -> p h f", h=H),
                in1=ex_st[:, 8 * j:8 * j + 8].unsqueeze(2).to_broadcast([128, H, F]),
                op=mybir.AluOpType.mult)
            first = ch == meta["blk_first"][cur_blk]
            last = ch == meta["blk_last"][cur_blk]
            nc.tensor.matmul(num_ps[:], lhsT=S[:], rhs=msg[:], start=first, stop=last)
            nc.tensor.matmul(den_ps[:], lhsT=S[:], rhs=ex_st[:, 8 * j:8 * j + 8],
                             start=first, stop=last)
            if last:
                epil(cur_blk, num_ps, den_ps)


def _load_edge_consts(nc, tc, ctx, meta, inp):
    consts = {}
    cp = ctx.enter_context(tc.tile_pool(name="econst", bufs=1))
    IC = sum(nch * 8 for (_, _, nch) in meta["gathers"])
    isrc_sb = cp.tile([128, IC], I16)
    nc.sync.dma_start(isrc_sb[:], inp["idx_src"][:])
    idst_sb = cp.tile([128, IC], I16)
    nc.sync.dma_start(idst_sb[:], inp["idx_dst"][:])
    dstl_sb = cp.tile([128, meta["TOTCH"]], F32)
    nc.sync.dma_start(dstl_sb[:], inp["dstl"][:])
    iota_bf = cp.tile([128, 128], BF16)
    nc.sync.dma_start(iota_bf[:], inp["iota_bf"][:])
    consts.update(isrc_sb=isrc_sb, idst_sb=idst_sb, dstl_sb=dstl_sb, iota_bf=iota_bf)
    return consts


def _build_merged(meta):
    nc = bacc.Bacc("TRN2", target_bir_lowering=False, debug=False, num_devices=NCORES)
    IC = sum(nch * 8 for (_, _, nch) in meta["gathers"])
    groups = [list(range(NCORES))]

    i_xT = nc.dram_tensor("xTsh", [32, NSHP], BF16, kind="ExternalInput")
    i_w1 = nc.dram_tensor("w1cat", [32, 520], BF16, kind="ExternalInput")
    i_wad1 = nc.dram_tensor("wad1", [32, 8], BF16, kind="ExternalInput")
    i_b1 = nc.dram_tensor("b1rep", [128, 64], F32, kind="ExternalInput")
    i_w2 = nc.dram_tensor("w2cat", [64, 520], BF16, kind="ExternalInput")
    i_wad2 = nc.dram_tensor("wad2", [64, 8], BF16, kind="ExternalInput")
    i_b2 = nc.dram_tensor("b2rep", [128, 512], F32, kind="ExternalInput")
    i_isrc = nc.dram_tensor("idx_src", [128, IC], I16, kind="ExternalInput")
    i_idst = nc.dram_tensor("idx_dst", [128, IC], I16, kind="ExternalInput")
    i_dstl = nc.dram_tensor("dstl", [128, meta["TOTCH"]], F32, kind="ExternalInput")
    i_iota = nc.dram_tensor("iota_bf", [128, 128], BF16, kind="ExternalInput")
    i_glw = nc.dram_tensor("glw", [128, NBLK * NWIN], F32, kind="ExternalInput")
    i_gridx = nc.dram_tensor("gridx", [128, GRID // 16], I16, kind="ExternalInput")
    i_fw1 = nc.dram_tensor("fcw1", [512, 512], BF16, kind="ExternalInput")
    i_fw2 = nc.dram_tensor("fcw2", [512, 512], BF16, kind="ExternalInput")
    i_fw3 = nc.dram_tensor("fcw3", [128, 4], BF16, kind="ExternalInput")
    i_fb1 = nc.dram_tensor("fcb1", [128, 4], F32, kind="ExternalInput")
    i_fb2 = nc.dram_tensor("fcb2", [128, 4], F32, kind="ExternalInput")
    i_fb3 = nc.dram_tensor("fcb3", [1, 1], F32, kind="ExternalInput")
    o_out = nc.dram_tensor("out", [1, GSH], F32, kind="ExternalOutput")

    with tile.TileContext(nc, num_cores=NCORES) as tc:
        with ExitStack() as ctx:
            dram = ctx.enter_context(tc.tile_pool(name="dram", bufs=1, space="DRAM"))
            tabloc1 = dram.tile([NSHP, 640], BF16)
            tabfull1 = dram.tile([NPADN, 640], BF16, addr_space="Shared")
            adtab1 = dram.tile([NSHP, 128], BF16)
            tabloc2 = dram.tile([NSHP, 640], BF16)
            tabfull2 = dram.tile([NPADN, 640], BF16, addr_space="Shared")
            adtab2 = dram.tile([NSHP, 128], BF16)
            winbuf = dram.tile([WROWS, 512], F32)
            grid = dram.tile([GRID, 512], F32)
            gmine = dram.tile([GSH, 512], F32)

            cp = ctx.enter_context(tc.tile_pool(name="wconst", bufs=1))
            x_sb = cp.tile([32, NSHP], BF16)
            nc.sync.dma_start(x_sb[:], i_xT[:])
            w1_sb = cp.tile([32, 520], BF16)
            nc.sync.dma_start(w1_sb[:], i_w1[:])
            wad1_sb = cp.tile([32, 8], BF16)
            nc.sync.dma_start(wad1_sb[:], i_wad1[:])
            b1_sb = cp.tile([128, 64], F32)
            nc.sync.dma_start(b1_sb[:], i_b1[:])
            w2_sb = cp.tile([64, 520], BF16)
            nc.sync.dma_start(w2_sb[:], i_w2[:])
            wad2_sb = cp.tile([64, 8], BF16)
            nc.sync.dma_start(wad2_sb[:], i_wad2[:])
            b2_sb = cp.tile([128, 512], F32)
            nc.sync.dma_start(b2_sb[:], i_b2[:])
            glw_sb = cp.tile([128, NBLK * NWIN], F32)
            nc.sync.dma_start(glw_sb[:], i_glw[:])
            gridx_sb = cp.tile([128, GRID // 16], I16)
            nc.sync.dma_start(gridx_sb[:], i_gridx[:])
            ident_bf = cp.tile([128, 128], BF16)
            make_identity(nc, ident_bf[:])
            h1own = cp.tile([128, NBLK * 64], BF16)
            consts = _load_edge_consts(nc, tc, ctx, meta, dict(
                idx_src=i_isrc, idx_dst=i_idst, dstl=i_dstl, iota_bf=i_iota))

            # ---------------- layer-1 table (own shard only) + AllGather
            with ExitStack() as tctx:
                ps5 = tctx.enter_context(tc.tile_pool(name="t1p5", bufs=2, space="PSUM"))
                ps8 = tctx.enter_context(tc.tile_pool(name="t1p8", bufs=2, space="PSUM"))
                psA = tctx.enter_context(tc.tile_pool(name="t1pa", bufs=2, space="PSUM"))
                rowp = tctx.enter_context(tc.tile_pool(name="t1row", bufs=3))
                for b in range(NBLK):
                    lhsT = x_sb[:, b * 128:(b + 1) * 128]
                    hps = ps5.tile([128, 512], F32, tag="hps")
                    nc.tensor.matmul(hps[:], lhsT=lhsT, rhs=w1_sb[:, 0:512],
                                     start=True, stop=True)
                    aps = ps8.tile([128, 8], F32, tag="aps")
                    nc.tensor.matmul(aps[:], lhsT=lhsT, rhs=w1_sb[:, 512:520],
                                     start=True, stop=True)
                    row = rowp.tile([128, 640], BF16, tag="row")
                    if b % 2 == 0:
                        nc.scalar.copy(row[:, 0:512], hps[:])
                    else:
                        nc.vector.tensor_copy(row[:, 0:512], hps[:])
                    nc.vector.tensor_copy(row[:, 512:520], aps[:])
                    nc.sync.dma_start(tabloc1[b * 128:(b + 1) * 128, 0:520],
                                      row[:, 0:520])
                    dps = psA.tile([128, 8], F32, tag="dps")
                    nc.tensor.matmul(dps[:], lhsT=lhsT, rhs=wad1_sb[:, 0:8],
                                     start=True, stop=True)
                    adrow = rowp.tile([128, 128], BF16, tag="adrow")
                    nc.vector.tensor_copy(adrow[:, 0:8], dps[:])
                    nc.sync.dma_start(adtab1[b * 128:(b + 1) * 128, :], adrow[:])

            nc.gpsimd.collective_compute(
                "AllGather", mybir.AluOpType.bypass, replica_groups=groups,
                ins=[tabloc1[:].opt()], outs=[tabfull1[:].opt()])

            # ---------------- layer-1 edge pass -> h1own in SBUF
            with ExitStack() as ectx:
                ep = ectx.enter_context(tc.tile_pool(name="epil1", bufs=3))

                def epil1(b, num_ps, den_ps):
                    den = ep.tile([128, 8], F32, tag="den")
                    nc.vector.tensor_scalar(out=den[:], in0=den_ps[:], scalar1=8.0,
                                            scalar2=1e-20, op0=mybir.AluOpType.mult,
                                            op1=mybir.AluOpType.add)
                    rec = ep.tile([128, 8], F32, tag="rec")
                    nc.vector.reciprocal(rec[:], den[:])
                    tmp = ep.tile([128, 512], F32, tag="tmp")
                    nc.vector.tensor_tensor(
                        out=tmp[:].rearrange("p (h f) -> p h f", h=H),
                        in0=num_ps[:].rearrange("p (h f) -> p h f", h=H),
                        in1=rec[:].unsqueeze(2).to_broadcast([128, H, F]),
                        op=mybir.AluOpType.mult)
                    t3 = tmp[:].rearrange("p (h f) -> p h f", h=H)
                    a4 = ep.tile([128, 256], F32, tag="a4")
                    nc.vector.tensor_tensor(
                        out=a4[:].rearrange("p (h f) -> p h f", h=4),
                        in0=t3[:, 0:4, :], in1=t3[:, 4:8, :], op=mybir.AluOpType.add)
                    a4v = a4[:].rearrange("p (h f) -> p h f", h=4)
                    a2 = ep.tile([128, 128], F32, tag="a2")
                    nc.vector.tensor_tensor(
                        out=a2[:].rearrange("p (h f) -> p h f", h=2),
                        in0=a4v[:, 0:2, :], in1=a4v[:, 2:4, :], op=mybir.AluOpType.add)
                    a2v = a2[:].rearrange("p (h f) -> p h f", h=2)
                    a1 = ep.tile([128, 64], F32, tag="a1")
                    nc.vector.tensor_tensor(out=a1[:], in0=a2v[:, 0, :], in1=a2v[:, 1, :],
                                            op=mybir.AluOpType.add)
                    nc.vector.tensor_tensor(out=h1own[:, b * 64:(b + 1) * 64],
                                            in0=a1[:], in1=b1_sb[:],
                                            op=mybir.AluOpType.add)

                _edge_pass(nc, tc, ectx, meta, tabfull1[0:HALF, :],
                           tabfull1[HALF:NPADN, :], adtab1, 1, consts, epil1)

            # ---------------- layer-2 table (own shard only) + AllGather
            with ExitStack() as tctx:
                ps5 = tctx.enter_context(tc.tile_pool(name="t2p5", bufs=2, space="PSUM"))
                ps8 = tctx.enter_context(tc.tile_pool(name="t2p8", bufs=2, space="PSUM"))
                psA = tctx.enter_context(tc.tile_pool(name="t2pa", bufs=2, space="PSUM"))
                psT = tctx.enter_context(tc.tile_pool(name="t2pt", bufs=2, space="PSUM"))
                rowp = tctx.enter_context(tc.tile_pool(name="t2row", bufs=3))
                htp = tctx.enter_context(tc.tile_pool(name="t2ht", bufs=2))
                for b in range(NBLK):
                    tps = psT.tile([64, 128], BF16, tag="tps")
                    nc.tensor.transpose(tps[:], h1own[:, b * 64:(b + 1) * 64],
                                        ident_bf[:])
                    hT = htp.tile([64, 128], BF16, tag="hT")
                    nc.vector.tensor_copy(hT[:], tps[:])
                    hps = ps5.tile([128, 512], F32, tag="hps")
                    nc.tensor.matmul(hps[:], lhsT=hT[:], rhs=w2_sb[:, 0:512],
                                     start=True, stop=True)
                    aps = ps8.tile([128, 8], F32, tag="aps")
                    nc.tensor.matmul(aps[:], lhsT=hT[:], rhs=w2_sb[:, 512:520],
                                     start=True, stop=True)
                    row = rowp.tile([128, 640], BF16, tag="row")
                    if b % 2 == 0:
                        nc.scalar.copy(row[:, 0:512], hps[:])
                    else:
                        nc.vector.tensor_copy(row[:, 0:512], hps[:])
                    nc.vector.tensor_copy(row[:, 512:520], aps[:])
                    nc.sync.dma_start(tabloc2[b * 128:(b + 1) * 128, 0:520],
                                      row[:, 0:520])
                    dps = psA.tile([128, 8], F32, tag="dps")
                    nc.tensor.matmul(dps[:], lhsT=hT[:], rhs=wad2_sb[:, 0:8],
                                     start=True, stop=True)
                    adrow = rowp.tile([128, 128], BF16, tag="adrow")
                    nc.vector.tensor_copy(adrow[:, 0:8], dps[:])
                    nc.sync.dma_start(adtab2[b * 128:(b + 1) * 128, :], adrow[:])

            nc.gpsimd.collective_compute(
                "AllGather", mybir.AluOpType.bypass, replica_groups=groups,
                ins=[tabloc2[:].opt()], outs=[tabfull2[:].opt()])

            # ---------------- layer-2 edge pass + window pooling
            with ExitStack() as ectx:
                ep = ectx.enter_context(tc.tile_pool(name="epil2", bufs=3))
                sgp = ectx.enter_context(tc.tile_pool(name="sg", bufs=3))
                psG = ectx.enter_context(tc.tile_pool(name="psg", bufs=1, space="PSUM"))
                gw_ps = []
                for w in range(NWIN):
                    gw_tile = psG.tile([128, 512], F32, tag=f"gw{w}")
                    gw_ps.append(gw_tile)

                def epil2(b, num_ps, den_ps):
                    den = ep.tile([128, 8], F32, tag="den")
                    nc.vector.tensor_scalar(out=den[:], in0=den_ps[:], scalar1=1e-20,
                                            scalar2=None, op0=mybir.AluOpType.add)
                    rec = ep.tile([128, 8], F32, tag="rec")
                    nc.vector.reciprocal(rec[:], den[:])
                    o2f = ep.tile([128, 512], F32, tag="o2f")
                    nc.vector.tensor_tensor(
                        out=o2f[:].rearrange("p (h f) -> p h f", h=H),
                        in0=num_ps[:].rearrange("p (h f) -> p h f", h=H),
                        in1=rec[:].unsqueeze(2).to_broadcast([128, H, F]),
                        op=mybir.AluOpType.mult)
                    o2 = ep.tile([128, 512], BF16, tag="o2")
                    nc.vector.tensor_tensor(out=o2[:], in0=o2f[:], in1=b2_sb[:],
                                            op=mybir.AluOpType.add)
                    for w in range(NWIN):
                        Sg = sgp.tile([128, 128], BF16, tag="Sg")
                        nc.vector.tensor_scalar(
                            out=Sg[:], in0=consts["iota_bf"][:],
                            scalar1=glw_sb[:, b * NWIN + w:b * NWIN + w + 1],
                            scalar2=None, op0=mybir.AluOpType.is_equal)
                        nc.tensor.matmul(gw_ps[w][:], lhsT=Sg[:], rhs=o2[:],
                                         start=(b == 0), stop=(b == NBLK - 1))

                _edge_pass(nc, tc, ectx, meta, tabfull2[0:HALF, :],
                           tabfull2[HALF:NPADN, :], adtab2, 2, consts, epil2)

                # windows -> winbuf rows [0, 384); zero rows [384, 512)
                zt = ep.tile([128, 512], F32, tag="zt")
                nc.gpsimd.memset(zt[:], 0.0)
                nc.sync.dma_start(winbuf[NWIN * 128:WROWS, :], zt[:])
                for w in range(NWIN):
                    wsb = ep.tile([128, 512], F32, tag="wsb")
                    nc.vector.tensor_copy(wsb[:], gw_ps[w][:])
                    nc.sync.dma_start(winbuf[w * 128:(w + 1) * 128, :], wsb[:])

            # ---------------- grid assembly + ReduceScatter
            with ExitStack() as gctx:
                gp = gctx.enter_context(tc.tile_pool(name="gridp", bufs=3))
                for t in range(GRID // 128):
                    gtile = gp.tile([128, 1, 512], F32, tag="gtile")
                    nc.gpsimd.dma_gather(
                        out_ap=gtile[:], in_ap=winbuf[:],
                        idxs_ap=gridx_sb[:, t * 8:(t + 1) * 8],
                        num_idxs=128, num_idxs_reg=128, elem_size=512)
                    nc.sync.dma_start(grid[t * 128:(t + 1) * 128, :],
                                      gtile[:, 0, :])

            nc.gpsimd.collective_compute(
                "ReduceScatter", mybir.AluOpType.add, replica_groups=groups,
                ins=[grid[:].opt()], outs=[gmine[:].opt()])

            # ---------------- MLP on own 256 graphs
            with ExitStack() as mctx:
                mw = mctx.enter_context(tc.tile_pool(name="mw", bufs=1))
                fw1, fw2 = [], []
                for k in range(4):
                    fw1_t = mw.tile([128, 512], BF16, tag=f"fw1{k}")
                    fw1.append(fw1_t)
                    fw2_t = mw.tile([128, 512], BF16, tag=f"fw2{k}")
                    fw2.append(fw2_t)
                for k in range(4):
                    nc.sync.dma_start(fw1[k][:], i_fw1[k * 128:(k + 1) * 128, :])
                    nc.sync.dma_start(fw2[k][:], i_fw2[k * 128:(k + 1) * 128, :])
                fw3 = mw.tile([128, 4], BF16)
                nc.sync.dma_start(fw3[:], i_fw3[:])
                fb1 = mw.tile([128, 4], F32)
                nc.sync.dma_start(fb1[:], i_fb1[:])
                fb2 = mw.tile([128, 4], F32)
                nc.sync.dma_start(fb2[:], i_fb2[:])
                fb3 = mw.tile([1, 1], F32)
                nc.sync.dma_start(fb3[:], i_fb3[:])
                ident_f = mw.tile([128, 128], F32)
                make_identity(nc, ident_f[:])

                gp = mctx.enter_context(tc.tile_pool(name="mg", bufs=2))
                psT = mctx.enter_context(tc.tile_pool(name="mpt", bufs=2, space="PSUM"))
                psA = mctx.enter_context(tc.tile_pool(name="mpa", bufs=2, space="PSUM"))
                psO = mctx.enter_context(tc.tile_pool(name="mpo", bufs=2, space="PSUM"))
                ap_ = mctx.enter_context(tc.tile_pool(name="ma", bufs=2))

                for gt in range(GSH // 128):
                    gl = gp.tile([128, 512], F32, tag="gl")
                    nc.sync.dma_start(gl[:], gmine[gt * 128:(gt + 1) * 128, :])
                    gTs = []
                    for k in range(4):
                        tps = psT.tile([128, 128], F32, tag="tps")
                        nc.tensor.transpose(tps[:], gl[:, k * 128:(k + 1) * 128],
                                            ident_f[:])
                        gT = ap_.tile([128, 128], BF16, tag=f"gT{k}")
                        nc.vector.tensor_copy(gT[:], tps[:])
                        gTs.append(gT)
                    a1s, a2s = [], []
                    for m in range(4):
                        aps = psA.tile([128, 128], F32, tag="aps")
                        for k in range(4):
                            nc.tensor.matmul(aps[:], lhsT=fw1[k][:, m * 128:(m + 1) * 128],
                                             rhs=gTs[k][:], start=(k == 0), stop=(k == 3))
                        a1 = ap_.tile([128, 128], BF16, tag=f"a1{m}")
                        nc.scalar.activation(a1[:], aps[:],
                                             mybir.ActivationFunctionType.Relu,
                                             bias=fb1[:, m:m + 1])
                        a1s.append(a1)
                    for m in range(4):
                        aps = psA.tile([128, 128], F32, tag="bps")
                        for k in range(4):
                            nc.tensor.matmul(aps[:], lhsT=fw2[k][:, m * 128:(m + 1) * 128],
                                             rhs=a1s[k][:], start=(k == 0), stop=(k == 3))
                        a2 = ap_.tile([128, 128], BF16, tag=f"a2{m}")
                        nc.scalar.activation(a2[:], aps[:],
                                             mybir.ActivationFunctionType.Relu,
                                             bias=fb2[:, m:m + 1])
                        a2s.append(a2)
                    ops = psO.tile([128, 128], F32, tag="ops")
                    for k in range(4):
                        nc.tensor.matmul(ops[0:1, :], lhsT=fw3[:, k:k + 1], rhs=a2s[k][:],
                                         start=(k == 0), stop=(k == 3))
                    osb = ap_.tile([128, 128], F32, tag="osb")
                    nc.scalar.activation(osb[0:1, :], ops[0:1, :],
                                         mybir.ActivationFunctionType.Identity,
                                         bias=fb3[0:1, 0:1])
                    nc.sync.dma_start(o_out[0:1, gt * 128:(gt + 1) * 128], osb[0:1, :])

    nc.compile()
    return nc


# ------------------------------------------------- cached PJRT runner
def _ensure_runner(nc):
    """Build the jitted shard_map executor for nc (once per program)."""
    import jax
    from jax.sharding import Mesh, PartitionSpec, NamedSharding
    from jax.experimental.shard_map import shard_map
    from concourse import bass2jax
    from concourse.bass2jax import _bass_exec_p, partition_id_tensor

    st = _cache.setdefault("runner", {})
    if "fn" in st:
        return st
    bass2jax.install_neuronx_cc_hook()
    partition_name = (nc.partition_id_tensor.name
                      if nc.partition_id_tensor else None)
    in_names, out_names, out_avals = [], [], []
    for alloc in nc.m.functions[0].allocations:
        if not isinstance(alloc, mybir.MemoryLocationSet):
            continue
        name = alloc.memorylocations[0].name
        if alloc.kind == "ExternalInput":
            if name != partition_name:
                in_names.append(name)
        elif alloc.kind == "ExternalOutput":
            shape = tuple(alloc.tensor_shape)
            dtype = mybir.dt.np(alloc.dtype)
            out_names.append(name)
            out_avals.append(jax.core.ShapedArray(shape, dtype))
    n_params = len(in_names)
    all_names = list(in_names) + list(out_names)
    if partition_name is not None:
        all_names.append(partition_name)
    donate = tuple(range(n_params, n_params + len(out_names)))

    def _body(*args):
        operands = list(args)
        if partition_name is not None:
            operands.append(partition_id_tensor())
        outs = _bass_exec_p.bind(
            *operands, out_avals=tuple(out_avals), in_names=tuple(all_names),
            out_names=tuple(out_names), lowering_input_output_aliases=(),
            sim_require_finite=True, sim_require_nnan=True, nc=nc)
        return tuple(outs)

    devices = jax.devices()[:NCORES]
    mesh = Mesh(np.asarray(devices), ("core",))
    spec_in = (PartitionSpec("core"),) * (n_params + len(out_names))
    spec_out = (PartitionSpec("core"),) * len(out_names)
    fn = jax.jit(shard_map(_body, mesh=mesh, in_specs=spec_in,
                           out_specs=spec_out, check_rep=False),
                 donate_argnums=donate, keep_unused=True)
    st.update(fn=fn, in_names=in_names, out_names=out_names,
              out_avals=out_avals,
              shard=NamedSharding(mesh, PartitionSpec("core")))
    return st


# ----------------------------------------------------------------- kernel()
def kernel(x, edge_index, batch, W1, a_src1, a_dst1, b1, W2, a_src2, a_dst2, b2,
           fcW1, fcb1, fcW2, fcb2, fcW3, fcb3):
    import jax

    raws = (x, edge_index, batch, W1, a_src1, a_dst1, b1, W2, a_src2, a_dst2,
            b2, fcW1, fcb1, fcW2, fcb2, fcW3, fcb3)
    raws = tuple(np.asarray(a) for a in raws)
    cached = _cache.get("raws")
    same = [cached is not None and len(cached) == len(raws)
            and a.shape == b.shape and np.array_equal(a, b)
            for a, b in zip(raws, cached or raws)]
    hit = bool(same) and all(same)

    if not hit:
        graph_same = bool(same) and same[1] and same[2] and "meta" in _cache
        _cache["raws"] = tuple(np.array(a, copy=True) for a in raws)
        (x, edge_index, batch, W1, a_src1, a_dst1, b1, W2, a_src2, a_dst2,
         b2, fcW1, fcb1, fcW2, fcb2, fcW3, fcb3) = raws
        x = np.asarray(x, np.float32)

        meta = _cache["meta"] if graph_same else _preprocess(edge_index, batch)
        _cache["meta"] = meta
        key = (meta["TOTCH"], len(meta["gathers"]), tuple(meta["g0"]))
        if _cache.get("progkey") != key:
            _cache["prog"] = _build_merged(meta)
            _cache["progkey"] = key
            _cache.pop("runner", None)

        # host-side constant prep
        xpad = np.zeros((NPADN, NF), np.float32)
        for c in range(NCORES):
            xpad[c * NSHP:c * NSHP + NSH] = x[c * NSH:(c + 1) * NSH]
        w1cat = np.concatenate(
            [np.asarray(W1, np.float32),
             _wcat(np.asarray(W1, np.float32), np.asarray(a_src1, np.float32))],
            axis=1).astype(BF)
        wad1 = _wcat(np.asarray(W1, np.float32),
                     np.asarray(a_dst1, np.float32)).astype(BF)
        W2f = np.asarray(W2, np.float32)
        w2cat = np.concatenate([W2f, _wcat(W2f, np.asarray(a_src2, np.float32))],
                               axis=1).astype(BF)
        wad2 = _wcat(W2f, np.asarray(a_dst2, np.float32)).astype(BF)
        b1rep = np.tile(np.asarray(b1, np.float32)[None, :], (128, 1))
        b2rep = np.tile(np.asarray(b2, np.float32)[None, :], (128, 1))
        iota_bf = np.tile(np.arange(128, dtype=np.float32), (128, 1)).astype(BF)
        fcb1a = np.asarray(fcb1, np.float32).reshape(4, 128).T.copy()
        fcb2a = np.asarray(fcb2, np.float32).reshape(4, 128).T.copy()
        fw3a = np.asarray(fcW3, np.float32).reshape(4, 128).T.astype(BF).copy()

        in_maps = []
        for c in range(NCORES):
            xTsh = np.ascontiguousarray(xpad[c * NSHP:(c + 1) * NSHP].T).astype(BF)
            in_maps.append(dict(
                xTsh=xTsh, w1cat=w1cat, wad1=wad1, b1rep=b1rep,
                w2cat=w2cat, wad2=wad2, b2rep=b2rep,
                idx_src=meta["idx_src"][c], idx_dst=meta["idx_dst"][c],
                dstl=meta["dstl"][c], iota_bf=iota_bf, glw=meta["glw"][c],
                gridx=meta["gridx"][c],
                fcw1=np.asarray(fcW1, np.float32).astype(BF),
                fcw2=np.asarray(fcW2, np.float32).astype(BF), fcw3=fw3a,
                fcb1=fcb1a, fcb2=fcb2a,
                fcb3=np.asarray(fcb3, np.float32).reshape(1, 1)))

        st = _ensure_runner(_cache["prog"])
        st["dev_args"] = [
            jax.device_put(
                np.concatenate([np.asarray(m[name]) for m in in_maps], axis=0),
                st["shard"])
            for name in st["in_names"]]

    st = _cache["runner"]
    zeros = [jax.device_put(
        np.zeros((NCORES * av.shape[0], *av.shape[1:]), av.dtype), st["shard"])
        for av in st["out_avals"]]

    t0 = time.time()
    outs = st["fn"](*st["dev_args"], *zeros)
    res = [np.asarray(o) for o in outs]
    LAST_TIMES.clear()
    LAST_TIMES["p"] = time.time() - t0

    oi = st["out_names"].index("out")
    out = res[oi].reshape(NCORES, GSH).reshape(-1)  # [2048] in core order
    return out[:G].astype(np.float32).reshape(G, 1)
